# revision 15
# baseline (speedup 1.0000x reference)
"""HRR adapted attention kernel for 8 trn2 cores.

Math (verified vs reference in f64):
  q,k,v = h @ W{q,k,v}.T + b      (per-row, D=2048)
  Qf = rfft(q); Kf = rfft(k)/(|rfft(k)|+eps); Vf likewise
  Mf = causal-cumsum_S(Kf*Vf);  Of = conj(Qf)*Mf;  adapter = irfft(Of)
  out = base + gate*adapter

All FFTs become matmuls: the DFT folds into the projections,
G = W.T @ [C|S] in [d,f] orientation, so the Q/K/V spectra come straight
out of hT.T @ G in a freq-on-partition layout where the causal cumsum is
a native tensor_tensor_scan along the free (sequence) dim.

Sharding: rows (B*S=8192) split 1024/core; the fold is tensor-parallel
over d (256 cols/core) + 6 AllGathers; cross-core cumsum offsets via a
tiny grouped AllGather of per-core block sums.
"""

import numpy as np

import concourse.bass as bass
import concourse.mybir as mybir
import concourse.tile as tile
from concourse import bacc, bass_utils
from concourse.masks import make_identity

F32 = mybir.dt.float32
F32R = mybir.dt.float32r
AX = mybir.AxisListType
ALU = mybir.AluOpType
ACTF = mybir.ActivationFunctionType

B, S, D = 2, 4096, 2048
R = B * S                  # 8192 flat rows
N_CORES = 8
SC = R // N_CORES          # 1024 rows per core
DC = D // N_CORES          # 256 d-columns per core (fold shard)
F = D // 2 + 1             # 1025 rfft bins
# Packed spectrum: col 0 re-plane = DC, col 0 im-plane = Nyquist (both are
# real bins), cols 1..1023 = bins 1..1023 -> exactly 8 tiles of 128.
FP = 1024
NFT = FP // 128            # 8 freq tiles
ND = D // 128              # 16 d tiles
NE = D // 128              # 16 e tiles
NST = SC // 128            # 8 row tiles per core
EPS = 1e-8
FOLD_CHUNKS = [(0, 512), (512, 512)]   # all >=256 for fp32r rate
# mat order; (name, use_sin(ci), w_idx, bias_col)
MATS = [("kre", 0, 1, 2), ("kim", 1, 1, 3),
        ("vre", 0, 2, 4), ("vim", 1, 2, 5),
        ("qre", 0, 0, 0), ("qim", 1, 0, 1)]

_CACHE = {}


def _r(ap):
    return ap.bitcast(F32R)


def _build():
    nc = bacc.Bacc("TRN2", target_bir_lowering=False, debug=False,
                   enable_asserts=False, num_devices=N_CORES)

    h_in = nc.dram_tensor("h", [SC, D], F32, kind="ExternalInput").ap()
    base_in = nc.dram_tensor("base", [SC, D], F32, kind="ExternalInput").ap()
    w_ins = [nc.dram_tensor(f"w{x}", [D, DC], F32, kind="ExternalInput").ap()
             for x in "qkv"]
    cp_in = nc.dram_tensor("cp", [D, FP], F32, kind="ExternalInput").ap()
    sp_in = nc.dram_tensor("sp", [D, FP], F32, kind="ExternalInput").ap()
    am_in = nc.dram_tensor("am", [FP, D], F32, kind="ExternalInput").ap()
    bm_in = nc.dram_tensor("bm", [FP, D], F32, kind="ExternalInput").ap()
    bf_in = nc.dram_tensor("bf", [FP, 6], F32, kind="ExternalInput").ap()
    mask_in = nc.dram_tensor("maskm", [128, 4], F32, kind="ExternalInput").ap()
    gate_in = nc.dram_tensor("gatec", [128, 1], F32, kind="ExternalInput").ap()
    out_t = nc.dram_tensor("out", [SC, D], F32, kind="ExternalOutput").ap()

    with tile.TileContext(nc) as tc, \
         tc.tile_pool(name="pc", bufs=1) as PC, \
         tc.tile_pool(name="psum", bufs=1, space="PSUM") as PP, \
         tc.tile_pool(name="dram", bufs=1, space="DRAM") as DR:

        # ---------- constants ----------
        ident = PC.tile([128, 128], F32, tag="ident")
        make_identity(nc, ident[:])
        mask_sb = PC.tile([128, 4], F32, tag="mask")
        nc.sync.dma_start(mask_sb[:], mask_in[:])
        gate_sb = PC.tile([128, 1], F32, tag="gate")
        nc.sync.dma_start(gate_sb[:], gate_in[:])
        zeros_sb = PC.tile([128, SC], F32, tag="zeros")
        nc.vector.memset(zeros_sb[:], 0.0)
        eps_sb = PC.tile([128, 1], F32, tag="eps")
        nc.vector.memset(eps_sb[:], EPS * EPS)
        bf_sb = PC.tile([128, NFT * 6], F32, tag="bf")
        nc.sync.dma_start(bf_sb[:].rearrange("p (t c) -> p t c", c=6),
                          bf_in.rearrange("(t p) c -> p t c", p=128))

        # ---------- DRAM intermediates ----------
        gin = [DR.tile([DC, FP], F32, tag=f"gin{m}", name=f"gin{m}") for m in range(6)]
        gout = [DR.tile([D, FP], F32, tag=f"gout{m}", name=f"gout{m}", addr_space="Shared") for m in range(6)]
        tin = DR.tile([2 * NFT, 128], F32, tag="tin")
        tout = DR.tile([4 * 2 * NFT, 128], F32, tag="tout")
        q_dram = [DR.tile([FP, SC], F32, tag=f"qd{p}", name=f"qd{p}") for p in range(2)]
        m_dram = [DR.tile([FP, SC], F32, tag=f"md{p}", name=f"md{p}") for p in range(2)]

        with tc.tile_pool(name="pht", bufs=1) as PH:
            hT = [PH.tile([128, SC], F32, tag=f"hT{d}", name=f"hT{d}") for d in range(ND)]

            # ============ fold + h transpose ============
            with tc.tile_pool(name="pf1", bufs=1) as PF1, \
                 tc.tile_pool(name="pf2", bufs=2) as PF2:
                for (f0, fw) in FOLD_CHUNKS:
                    cs_t = {}
                    for ci, cs_in in enumerate((cp_in, sp_in)):
                        for e in range(NE):
                            t = PF1.tile([128, 512], F32, tag=f"cs{ci}_{e}")
                            nc.sync.dma_start(
                                _r(t[:, :fw]),
                                _r(cs_in[e * 128:(e + 1) * 128, f0:f0 + fw]))
                            cs_t[(ci, e)] = t
                    for wi in range(3):
                        w_t = []
                        for e in range(NE):
                            t = PF1.tile([128, DC], F32, tag=f"wt{e}")
                            nc.sync.dma_start(
                                _r(t[:]), _r(w_ins[wi][e * 128:(e + 1) * 128, :]))
                            w_t.append(t)
                        for mi, (_, ci, wj, _) in enumerate(MATS):
                            if wj != wi:
                                continue
                            for dt in range(DC // 128):
                                psf = PP.tile([128, fw], F32, tag="foldp")
                                for e in range(NE):
                                    nc.tensor.matmul(
                                        psf[:],
                                        _r(w_t[e][:, dt * 128:(dt + 1) * 128]),
                                        _r(cs_t[(ci, e)][:, :fw]),
                                        start=(e == 0), stop=(e == NE - 1))
                                gtmp = PF2.tile([128, 512], F32, tag="gtmp")
                                nc.scalar.copy(gtmp[:, :fw], psf[:])
                                nc.sync.dma_start(
                                    gin[mi][dt * 128:(dt + 1) * 128, f0:f0 + fw],
                                    gtmp[:, :fw])

                # h transposes fill the PE while the AllGathers run
                for st in range(NST):
                    hstage = PF2.tile([128, D], F32, tag="hstage")
                    nc.sync.dma_start(hstage[:], h_in[st * 128:(st + 1) * 128, :])
                    for dt in range(ND):
                        pst = PP.tile([128, 128], F32, tag="trp")
                        nc.tensor.transpose(
                            pst[:], hstage[:, dt * 128:(dt + 1) * 128], ident[:])
                        nc.scalar.copy(_r(hT[dt][:, st * 128:(st + 1) * 128]),
                                       _r(pst[:]))

            for mi in range(6):
                nc.gpsimd.collective_compute(
                    "AllGather", ALU.bypass,
                    replica_groups=[list(range(N_CORES))],
                    ins=[gin[mi].opt()], outs=[gout[mi].opt()])

            # ============ projections + bind + scan ============
            with tc.tile_pool(name="pm1", bufs=1) as PM1, \
                 tc.tile_pool(name="pm2", bufs=2) as PM2:

                tots = PM1.tile([128, 2 * NFT], F32, tag="tots")
                planes = {}

                def proj(mi, fts):
                    bcol = MATS[mi][3]
                    psums = {ft: PP.tile([128, SC], F32, tag=f"pp{ft % 3}",
                                          name=f"pp{ft % 3}")
                             for ft in fts}
                    for dt in range(ND):
                        g3 = PM2.tile([128, 128 * len(fts)], F32, tag="g3")
                        nc.sync.dma_start(
                            _r(g3[:]), _r(gout[mi][dt * 128:(dt + 1) * 128,
                                          fts[0] * 128:(fts[-1] + 1) * 128]))
                        for j, ft in enumerate(fts):
                            for nh in range(2):
                                nc.tensor.matmul(
                                    psums[ft][:, nh * 512:(nh + 1) * 512],
                                    _r(g3[:, j * 128:(j + 1) * 128]),
                                    _r(hT[dt][:, nh * 512:(nh + 1) * 512]),
                                    start=(dt == 0), stop=(dt == ND - 1))
                    for ft in fts:
                        pl = PM1.tile([128, SC], F32, tag=f"pl{mi % 4}_{ft % 3}")
                        nc.scalar.activation(
                            pl[:], psums[ft][:], ACTF.Identity,
                            bias=bf_sb[:, ft * 6 + bcol:ft * 6 + bcol + 1])
                        planes[(mi, ft)] = pl

                def norm_bind_scan(ft):
                    kre, kim = planes[(0, ft)], planes[(1, ft)]
                    vre, vim = planes[(2, ft)], planes[(3, ft)]
                    rk = PM2.tile([128, SC], F32, tag="rk")
                    rv = PM2.tile([128, SC], F32, tag="rv")
                    t1 = PM2.tile([128, SC], F32, tag="t1")
                    t2 = PM2.tile([128, SC], F32, tag="t2")
                    for (re, im, rr) in ((kre, kim, rk), (vre, vim, rv)):
                        nc.scalar.square(t1[:], re[:])
                        nc.scalar.square(t2[:], im[:])
                        nc.vector.tensor_add(t1[:], t1[:], t2[:])
                        nc.scalar.activation(rr[:], t1[:], ACTF.Sqrt,
                                             bias=eps_sb[:])
                        nc.vector.reciprocal(rr[:], rr[:])
                    cre = PM2.tile([128, SC], F32, tag="kvre")
                    cim = PM2.tile([128, SC], F32, tag="kvim")
                    nc.vector.tensor_mul(t1[:], kre[:], vre[:])
                    nc.vector.tensor_mul(t2[:], kim[:], vim[:])
                    nc.vector.tensor_sub(cre[:], t1[:], t2[:])
                    nc.vector.tensor_mul(t1[:], kre[:], vim[:])
                    nc.vector.tensor_mul(t2[:], kim[:], vre[:])
                    nc.vector.tensor_add(cim[:], t1[:], t2[:])
                    nc.vector.tensor_mul(rk[:], rk[:], rv[:])
                    nc.vector.tensor_mul(cre[:], cre[:], rk[:])
                    nc.vector.tensor_mul(cim[:], cim[:], rk[:])
                    if ft == 0:
                        # partition row 0 holds two REAL bins (DC in re,
                        # Nyquist in im) -> normalize/bind each separately
                        r0 = []
                        for pl in (kre, kim, vre, vim):
                            rr0 = PM2.tile([1, SC], F32, tag="rr0", bufs=4)
                            nc.scalar.square(rr0[:], pl[0:1, :])
                            nc.scalar.activation(rr0[:], rr0[:], ACTF.Sqrt,
                                                 bias=eps_sb[0:1, :])
                            nc.vector.reciprocal(rr0[:], rr0[:])
                            r0.append(rr0)
                        nc.vector.tensor_mul(cre[0:1, :], kre[0:1, :], vre[0:1, :])
                        nc.vector.tensor_mul(cre[0:1, :], cre[0:1, :], r0[0][:])
                        nc.vector.tensor_mul(cre[0:1, :], cre[0:1, :], r0[2][:])
                        nc.vector.tensor_mul(cim[0:1, :], kim[0:1, :], vim[0:1, :])
                        nc.vector.tensor_mul(cim[0:1, :], cim[0:1, :], r0[1][:])
                        nc.vector.tensor_mul(cim[0:1, :], cim[0:1, :], r0[3][:])
                    # zero-init causal scan (cross-core offset added later);
                    # last column is this core's block total
                    for pi, cv in enumerate((cre, cim)):
                        mt = PM2.tile([128, SC], F32, tag=f"mt{pi}")
                        nc.vector.tensor_tensor_scan(
                            mt[:], cv[:], zeros_sb[:], 0.0, ALU.add, ALU.add)
                        c = 2 * ft + pi
                        nc.vector.tensor_copy(tots[:, c:c + 1], mt[:, SC - 1:SC])
                        nc.sync.dma_start(
                            m_dram[pi][ft * 128:(ft + 1) * 128, :], mt[:])

                FGRPS = [[0, 1, 2], [3, 4, 5], [6, 7]]
                for fts in FGRPS:
                    for mi in range(4):
                        proj(mi, fts)
                    for ft in fts:
                        norm_bind_scan(ft)
                    for mi in (4, 5):          # Q projections -> DRAM spill
                        proj(mi, fts)
                        for ft in fts:
                            nc.sync.dma_start(
                                q_dram[mi - 4][ft * 128:(ft + 1) * 128, :],
                                planes[(mi, ft)][:])

                nc.sync.dma_start(tin.rearrange("c p -> p c"), tots[:])
                nc.gpsimd.collective_compute(
                    "AllGather", ALU.bypass,
                    replica_groups=[[0, 1, 2, 3], [4, 5, 6, 7]],
                    ins=[tin.opt()], outs=[tout.opt()])

        # ============ Of = conj(Qf)*(M + offset), irfft, epilogue ============
        with tc.tile_pool(name="pl1", bufs=1) as PL1, \
             tc.tile_pool(name="pl2", bufs=2) as PL2:
            of = {}
            tout_v = tout.rearrange("(r c) p -> c p r", c=2 * NFT)
            for ft in range(NFT):
                qs, ms, off = [], [], []
                for pi in range(2):
                    q_t = PL2.tile([128, SC], F32, tag=f"qs{pi}")
                    m_t = PL2.tile([128, SC], F32, tag=f"ms{pi}")
                    nc.sync.dma_start(q_t[:], q_dram[pi][ft * 128:(ft + 1) * 128, :])
                    nc.sync.dma_start(m_t[:], m_dram[pi][ft * 128:(ft + 1) * 128, :])
                    c = 2 * ft + pi
                    g4 = PL2.tile([128, 4], F32, tag="g4")
                    nc.sync.dma_start(g4[:], tout_v[c])
                    o_t = PL2.tile([128, 1], F32, tag=f"off{pi}")
                    nc.vector.tensor_mul(g4[:], g4[:], mask_sb[:])
                    nc.vector.tensor_reduce(o_t[:], g4[:], AX.X, ALU.add)
                    nc.vector.tensor_scalar_add(m_t[:], m_t[:], o_t[:])
                    qs.append(q_t); ms.append(m_t)
                t1 = PL2.tile([128, SC], F32, tag="t1")
                t2 = PL2.tile([128, SC], F32, tag="t2")
                ore = PL1.tile([128, SC], F32, tag=f"ore{ft}")
                oim = PL1.tile([128, SC], F32, tag=f"oim{ft}")
                nc.vector.tensor_mul(t1[:], qs[0][:], ms[0][:])
                nc.vector.tensor_mul(t2[:], qs[1][:], ms[1][:])
                nc.vector.tensor_add(_r(ore[:]), t1[:], t2[:])
                nc.vector.tensor_mul(t1[:], qs[0][:], ms[1][:])
                nc.vector.tensor_mul(t2[:], qs[1][:], ms[0][:])
                nc.vector.tensor_sub(_r(oim[:]), t1[:], t2[:])
                if ft == 0:
                    nc.vector.tensor_mul(_r(ore[0:1, :]), qs[0][0:1, :],
                                         ms[0][0:1, :])
                    nc.vector.tensor_mul(_r(oim[0:1, :]), qs[1][0:1, :],
                                         ms[1][0:1, :])
                of[ft] = (ore, oim)

            for dt in range(ND):
                psi = PP.tile([128, SC], F32, tag="pp0")
                at = PL2.tile([128, NFT * 128], F32, tag="at")
                bt = PL2.tile([128, NFT * 128], F32, tag="bt")
                nc.sync.dma_start(
                    _r(at[:].rearrange("p (t d) -> p t d", t=NFT)),
                    _r(am_in[:, dt * 128:(dt + 1) * 128]
                       .rearrange("(t p) d -> p t d", p=128)))
                nc.sync.dma_start(
                    _r(bt[:].rearrange("p (t d) -> p t d", t=NFT)),
                    _r(bm_in[:, dt * 128:(dt + 1) * 128]
                       .rearrange("(t p) d -> p t d", p=128)))
                for ft in range(NFT):
                    ore, oim = of[ft]
                    for nh in range(2):
                        nc.tensor.matmul(
                            psi[:, nh * 512:(nh + 1) * 512],
                            _r(at[:, ft * 128:(ft + 1) * 128]),
                            _r(ore[:, nh * 512:(nh + 1) * 512]),
                            start=(ft == 0), stop=False)
                        nc.tensor.matmul(
                            psi[:, nh * 512:(nh + 1) * 512],
                            _r(bt[:, ft * 128:(ft + 1) * 128]),
                            _r(oim[:, nh * 512:(nh + 1) * 512]),
                            start=False, stop=(ft == NFT - 1))
                adt = PL2.tile([128, SC], F32, tag="adT")
                nc.scalar.copy(adt[:], psi[:])
                for st in range(NST):
                    pst = PP.tile([128, 128], F32, tag="trp")
                    nc.tensor.transpose(
                        pst[:], adt[:, st * 128:(st + 1) * 128], ident[:])
                    btile = PL2.tile([128, 128], F32, tag="btile")
                    otile = PL2.tile([128, 128], F32, tag="otile")
                    nc.sync.dma_start(
                        btile[:], base_in[st * 128:(st + 1) * 128,
                                          dt * 128:(dt + 1) * 128])
                    nc.vector.scalar_tensor_tensor(
                        otile[:], pst[:], gate_sb[:], btile[:],
                        ALU.mult, ALU.add)
                    nc.sync.dma_start(
                        out_t[st * 128:(st + 1) * 128,
                              dt * 128:(dt + 1) * 128], otile[:])

    nc.compile()
    return nc


def _constants():
    e = np.arange(D, dtype=np.float64)
    f = np.arange(FP, dtype=np.float64)
    ang = 2.0 * np.pi * np.outer(e, f) / D           # [e, f]
    cp = np.cos(ang)
    sp = -np.sin(ang)
    sp[:, 0] = np.cos(np.pi * e)                     # Nyquist packed in im col 0
    w = np.full(FP, 2.0)
    w[0] = 1.0
    angA = 2.0 * np.pi * np.outer(f, e) / D          # [f, d]
    am = (w[:, None] / D) * np.cos(angA)
    bm = -(w[:, None] / D) * np.sin(angA)
    bm[0, :] = np.cos(np.pi * e) / D                 # Nyquist inverse row
    return (cp.astype(np.float32), sp.astype(np.float32),
            am.astype(np.float32), bm.astype(np.float32))


def _run(inputs, trace=False):
    if "nc" not in _CACHE:
        _CACHE["nc"] = _build()
    nc = _CACHE["nc"]
    cp, sp, am, bm = _CACHE.setdefault("const", _constants())

    h = np.ascontiguousarray(np.asarray(inputs["hidden_states"],
                                        np.float32).reshape(R, D))
    base = np.ascontiguousarray(np.asarray(inputs["base_output"],
                                           np.float32).reshape(R, D))
    gate = np.asarray(inputs["gate"], np.float32).reshape(-1)[0]

    bf = np.zeros((FP, 6), np.float32)
    for j, bn in enumerate(("bq", "bk", "bv")):
        spec = np.fft.rfft(np.asarray(inputs[bn], np.float64))
        bf[:FP, 2 * j] = spec.real[:FP].astype(np.float32)
        bf[:FP, 2 * j + 1] = spec.imag[:FP].astype(np.float32)
        bf[0, 2 * j + 1] = np.float32(spec.real[F - 1])
    gate_col = np.full((128, 1), gate, np.float32)

    ws = {x: np.asarray(inputs[f"W{x}"], np.float32) for x in "qkv"}
    in_maps = []
    for c in range(N_CORES):
        mask = np.zeros((128, 4), np.float32)
        mask[:, :c % 4] = 1.0
        in_maps.append({
            "h": h[c * SC:(c + 1) * SC],
            "base": base[c * SC:(c + 1) * SC],
            "wq": np.ascontiguousarray(ws["q"][:, c * DC:(c + 1) * DC]),
            "wk": np.ascontiguousarray(ws["k"][:, c * DC:(c + 1) * DC]),
            "wv": np.ascontiguousarray(ws["v"][:, c * DC:(c + 1) * DC]),
            "cp": cp, "sp": sp, "am": am, "bm": bm, "bf": bf,
            "maskm": mask, "gatec": gate_col,
        })

    res = bass_utils.run_bass_kernel_spmd(
        nc, in_maps, core_ids=list(range(N_CORES)), trace=trace)
    out = np.concatenate([res.results[c]["out"] for c in range(N_CORES)], axis=0)
    return out.reshape(B, S, D).astype(np.float32), res


def kernel(**inputs) -> np.ndarray:
    out, _ = _run(inputs, trace=False)
    return out
